# revision 7
# baseline (speedup 1.0000x reference)
"""BinaryLinear on 8 trn2 NeuronCores.

y = x @ sign(W).T + bias, x:(2,2048,4096) f32, W:(4096,4096) f32 [out,in],
bias:(4096,) f32.

Sharding: tensor-parallel over out_features — core c gets W rows
[c*512, (c+1)*512) and computes y[:, c*512:(c+1)*512] for all tokens.

The matmul stream runs in fp8-e4m3 DoubleRow perf mode (2 contraction
k-tiles per PE pass — ~1.9x the bf16 MAC rate measured). sign(W) is +-1,
exact in fp8; only the fp8 quantization of x adds error. To stay inside
the accuracy budget, x is encoded host-side as an fp8 pair stream
(layout/dtype marshalling only — all of the module's arithmetic stays on
device):
  - hi = e4m3(x) for all 4096 k-values,
  - lo = e4m3(x - hi) for the first KCV=1024 k-values (the "corrected"
    range; its quantization error cancels to ~2^-9 relative),
laid out as 40 k-tile slots per 128-token partition: slots 2j/2j+1 =
(hi_j, lo_j) for corrected k-tile j<8, slots 16..39 = hi_{8..31}. Each
DoubleRow matmul consumes one slot pair; corrected pairs reuse one sign
tile via a stride-0 broadcast lhsT, so W carries no duplicate slots.
W is shipped bf16 (fp8 host cast would flush ~26k tiny weights below the
e4m3 subnormal cutoff to sign 0) and binarized on device. Max rel err vs
the f32 reference is 1.85e-2 deterministic (quantization-dominated).

Device kernel (per core):
  - W^T bf16 arrives in k-quarters interleaved with the first token
    chunk's x slices on the sync HWDGE queue; sign() runs on ScalarE per
    quarter (bf16 in -> fp8 out), rotating across the 4 out-feature
    chunks. ScalarE does nothing else, so the signs clear the stream's
    deadlines.
  - matmul stream: per 512-token group, 4 psum banks (one per
    128-out-feature chunk) accumulate 20 DoubleRow matmuls each
    (lhsT = sign tile pair [128, 2, 128] fp8, rhs = x slot pair
    [128, 2, 512] fp8, fp32 accum); each x sub-load (2 pairs) feeds
    8 matmuls so the PE never outruns the DMA.
  - bias added on the (otherwise idle) DVE via tensor_scalar_add with a
    per-partition bias operand; fp32 y^T tiles DMA'd out on the ScalarE
    HWDGE queue.
A short chain of dummy matmuls on zeroed SBUF bridges the input-DMA
window so the PE's HAM clock gate is already ramped when the real
stream starts.
"""

import numpy as np
import ml_dtypes

B, S, D = 2, 2048, 4096
M = B * S            # 4096 tokens
NCORES = 8
NS = D // NCORES     # 512 out-features per core
P = 128
KO = D // P          # 32 contraction k-tiles
NC = NS // P         # 4 out-feature chunks per core
MB = 512             # tokens per matmul group (moving free dim)
MBL = 512            # tokens per x^T load chunk
MC = M // MBL        # 8 token load chunks

LC = 8               # corrected k-tiles (hi+lo residual pairs)
KCV = LC * P         # 1024 corrected k-values
NT = KO + LC         # 40 x-image slots (8 hi/lo pairs + 24 hi)
NPAIR = NT // 2      # 20 DoubleRow matmuls per (chunk, token group)
XSPLIT = 10          # x^T sub-loads per token chunk (4 slots each)
SPL = NT // XSPLIT   # 4 slots per sub-load
NQ = 4               # W load/sign quarters
QT = KO // NQ        # 8 k-tiles per quarter

E4 = ml_dtypes.float8_e4m3

_CACHE = {}


def _build():
    import concourse.mybir as mybir
    import concourse.tile as tile
    from concourse import bacc
    from concourse.bass import ts

    nc = bacc.Bacc("TRN2", target_bir_lowering=False, debug=False)

    # xt_img[mc, pi, t, mb]: fp8 slot image of x^T (see module docstring)
    xt_d = nc.dram_tensor(
        "xt_img", [MC, P, NT, MBL], mybir.dt.float8e4, kind="ExternalInput"
    )
    # wt_img[c, pi, ko, n] = bf16(W[c*128 + n, ko*128 + pi])
    wt_d = nc.dram_tensor(
        "wt_img", [NC, P, KO, P], mybir.dt.bfloat16, kind="ExternalInput"
    )
    bias_pc = nc.dram_tensor("bias_pc", [P, NC], mybir.dt.float32, kind="ExternalInput")
    yt_d = nc.dram_tensor("yt", [NS, M], mybir.dt.float32, kind="ExternalOutput")

    with tile.TileContext(nc) as tc:
        with (
            tc.tile_pool(name="const", bufs=1) as const_pool,
            tc.tile_pool(name="wtb", bufs=1) as wtb_pool,
            tc.tile_pool(name="wt8", bufs=1) as wt8_pool,
            tc.tile_pool(name="xt", bufs=2) as xt_pool,
            tc.tile_pool(name="yt", bufs=2) as yt_pool,
            tc.tile_pool(name="psum", bufs=2, space="PSUM") as psum_pool,
        ):
            # PE warm-up: dummy matmuls on zeroed SBUF fill the otherwise-
            # idle PE window during the input DMAs, so the HAM clock gate
            # is already ramped when the real matmul stream starts.
            warm = const_pool.tile([P, MB], mybir.dt.bfloat16)
            nc.gpsimd.memset(warm[:], 0)
            warm_ps = psum_pool.tile(
                [P, MB], mybir.dt.float32, tag="ps0", name="warm_ps"
            )
            NWARM = 7
            for i in range(NWARM):
                nc.tensor.matmul(
                    warm_ps[:], warm[:, :P], warm[:],
                    start=(i == 0), stop=(i == NWARM - 1),
                )

            wtbs = [
                wtb_pool.tile([P, KO, P], mybir.dt.bfloat16, name=f"wtb{c}")
                for c in range(NC)
            ]
            wt8s = [
                wt8_pool.tile([P, KO, P], mybir.dt.float8e4, name=f"wt8{c}")
                for c in range(NC)
            ]
            xs0 = [
                xt_pool.tile(
                    [P, SPL, MBL], mybir.dt.float8e4,
                    tag=f"xt{s}", name=f"xt{s}_0",
                )
                for s in range(XSPLIT)
            ]

            # sync-queue order: x slice 0, wt q0 (all chunks), x slice 1,
            # wt q1, ..., x slice 3, wt q3, x slices 4-9.  Each wt quarter
            # lands just before the matmul stream needs it; signs follow
            # the same rotation on ScalarE.
            def _load_wt_q(q):
                for c in range(NC):
                    nc.sync.dma_start(
                        wtbs[c][:, ts(q, QT), :], wt_d[c][:, ts(q, QT), :]
                    )

            def _load_x0(s):
                nc.sync.dma_start(xs0[s][:], xt_d[0][:, ts(s, SPL), :])

            _load_x0(0)
            _load_wt_q(0)
            _load_x0(1)
            _load_wt_q(1)
            _load_x0(2)
            _load_wt_q(2)
            _load_x0(3)
            _load_wt_q(3)
            for s in range(4, XSPLIT):
                _load_x0(s)

            # sign() split across three engines so the first token chunk's
            # later pairs aren't gated on one serial engine: ScalarE runs
            # chunks 0-1 (native Sign), DVE chunk 2 and GpSimd chunk 3 via
            # the two-pass clamp trick sign(w) = max(min(w*1e30, 1), -1)
            # (w is never exactly 0; +-1e30 saturates bf16 to +-inf).
            sgn_tmp = const_pool.tile([P, 2, QT, P], mybir.dt.bfloat16)
            for q in range(NQ):
                for c in (0, 1):
                    nc.scalar.activation(
                        wt8s[c][:, ts(q, QT), :],
                        wtbs[c][:, ts(q, QT), :],
                        mybir.ActivationFunctionType.Sign,
                    )
                for i, (eng, c) in enumerate(
                    ((nc.vector, 2), (nc.gpsimd, 3))
                ):
                    eng.tensor_scalar(
                        sgn_tmp[:, i], wtbs[c][:, ts(q, QT), :],
                        1e30, 1.0, mybir.AluOpType.mult, mybir.AluOpType.min,
                    )
                    eng.tensor_scalar_max(
                        wt8s[c][:, ts(q, QT), :], sgn_tmp[:, i], -1.0
                    )

            bias_sb = const_pool.tile([P, NC], mybir.dt.float32)
            nc.gpsimd.dma_start(bias_sb[:], bias_pc[:, :])

            def _lhsT(c, pr):
                if pr < LC:
                    # corrected pair: same sign tile for hi and lo
                    return wt8s[c][:, pr : pr + 1, :].broadcast_to([P, 2, P])
                u = LC + 2 * (pr - LC)
                return wt8s[c][:, u : u + 2, :]

            for mc in range(MC):
                if mc == 0:
                    xs = xs0
                else:
                    xs = []
                    for s in range(XSPLIT):
                        xt_s = xt_pool.tile(
                            [P, SPL, MBL], mybir.dt.float8e4, tag=f"xt{s}"
                        )
                        nc.sync.dma_start(xt_s[:], xt_d[mc][:, ts(s, SPL), :])
                        xs.append(xt_s)

                # Interleave the 4 psum groups over slot pairs: each x^T
                # sub-load (2 pairs) is consumed by all 4 out-feature
                # chunks before the next one is needed.
                pss = [
                    psum_pool.tile(
                        [P, MB], mybir.dt.float32,
                        tag=f"ps{c}", name=f"ps{c}_{mc}",
                    )
                    for c in range(NC)
                ]
                for s in range(XSPLIT):
                    for c in range(NC):
                        for pp in range(SPL // 2):
                            pr = s * (SPL // 2) + pp
                            nc.tensor.matmul(
                                pss[c][:],
                                _lhsT(c, pr),
                                xs[s][:, ts(pp, 2), :],
                                start=(pr == 0),
                                stop=(pr == NPAIR - 1),
                                perf_mode=mybir.MatmulPerfMode.DoubleRow,
                            )
                for c in range(NC):
                    yt = yt_pool.tile(
                        [P, MB], mybir.dt.float32,
                        tag=f"yt{c}", name=f"yt{c}_{mc}",
                    )
                    # GpSimd can't read PSUM, so split the bias-add
                    # between DVE and ScalarE (Identity activation).
                    if c % 2 == 0:
                        nc.vector.tensor_scalar_add(
                            yt[:], pss[c][:], bias_sb[:, c : c + 1]
                        )
                    else:
                        nc.scalar.activation(
                            yt[:],
                            pss[c][:],
                            mybir.ActivationFunctionType.Identity,
                            bias=bias_sb[:, c : c + 1],
                        )
                    nc.scalar.dma_start(yt_d[ts(c, P), ts(mc, MB)], yt[:])

    nc.compile()
    return nc


def _quantize_x(x):
    """x [M, D] f32 -> fp8 slot image [MC, P, NT, MBL].

    hi = e4m3(x) everywhere; lo = e4m3(x - hi) for the first KCV
    k-values (x - hi is exact in f32 by Sterbenz).
    """
    xt = np.ascontiguousarray(x.T)               # [D, M]
    hi = xt.astype(E4)
    res = xt - hi.astype(np.float32)
    lo = res[:KCV].astype(E4)

    slots = np.empty((NT, P, M), dtype=E4)
    hi_t = hi.reshape(KO, P, M)
    lo_t = lo.reshape(LC, P, M)
    slots[0 : 2 * LC : 2] = hi_t[:LC]
    slots[1 : 2 * LC : 2] = lo_t
    slots[2 * LC :] = hi_t[LC:]
    # [NT, P, MC, MBL] -> [MC, P, NT, MBL]
    img = slots.reshape(NT, P, MC, MBL).transpose(2, 1, 0, 3)
    return np.ascontiguousarray(img)


def _run(inputs, trace=False, **spmd_kwargs):
    from concourse.bass_utils import run_bass_kernel_spmd

    x = np.asarray(inputs["x"], dtype=np.float32).reshape(M, D)
    weight = np.asarray(inputs["weight"], dtype=np.float32)
    bias = np.asarray(inputs["bias"], dtype=np.float32)

    xt_img = _quantize_x(x)
    w_bf = weight.astype(ml_dtypes.bfloat16)
    in_maps = []
    for c in range(NCORES):
        # [NS, D] -> SBUF image [NC, pi, ko, n]
        w_c = w_bf[c * NS:(c + 1) * NS]
        wt_img = np.ascontiguousarray(
            w_c.reshape(NC, P, KO, P).transpose(0, 3, 2, 1)
        )
        b_pc = np.ascontiguousarray(
            bias[c * NS:(c + 1) * NS].reshape(NC, P).T
        )
        in_maps.append({"xt_img": xt_img, "wt_img": wt_img, "bias_pc": b_pc})

    if "nc" not in _CACHE:
        _CACHE["nc"] = _build()
    nc = _CACHE["nc"]

    res = run_bass_kernel_spmd(
        nc, in_maps, core_ids=list(range(NCORES)), trace=trace, **spmd_kwargs
    )
    # results[c]["yt"] is y[:, c*NS:(c+1)*NS].T — stack to y.T then transpose
    y_t = np.concatenate([res.results[c]["yt"] for c in range(NCORES)], axis=0)
    out = np.ascontiguousarray(y_t.T).reshape(B, S, D)
    return out, res


def kernel(**inputs) -> np.ndarray:
    out, _ = _run(inputs)
    return out


# revision 8
# speedup vs baseline: 1.6561x; 1.6561x over previous
"""BinaryLinear on 8 trn2 NeuronCores.

y = x @ sign(W).T + bias, x:(2,2048,4096) f32, W:(4096,4096) f32 [out,in],
bias:(4096,) f32.

Sharding: tensor-parallel over out_features — core c gets W rows
[c*512, (c+1)*512) and computes y[:, c*512:(c+1)*512] for all tokens.

The matmul stream runs in fp8-e4m3 DoubleRow perf mode (2 contraction
k-tiles per PE pass — ~1.9x the bf16 MAC rate measured). sign(W) is +-1,
exact in fp8; only the fp8 quantization of x adds error. To stay inside
the accuracy budget, x is encoded host-side as an fp8 pair stream
(layout/dtype marshalling only — all of the module's arithmetic stays on
device):
  - hi = e4m3(x) for all 4096 k-values,
  - lo = e4m3(x - hi) for the first KCV=1024 k-values (the "corrected"
    range; its quantization error cancels to ~2^-9 relative),
laid out as 40 k-tile slots per 128-token partition: slots 2j/2j+1 =
(hi_j, lo_j) for corrected k-tile j<8, slots 16..39 = hi_{8..31}. Each
DoubleRow matmul consumes one slot pair; corrected pairs reuse one sign
tile via a stride-0 broadcast lhsT, so W carries no duplicate slots.
W is shipped bf16 (fp8 host cast would flush ~26k tiny weights below the
e4m3 subnormal cutoff to sign 0) and binarized on device. Max rel err vs
the f32 reference is 1.85e-2 deterministic (quantization-dominated).

Device kernel (per core):
  - W^T bf16 arrives in k-quarters interleaved with the first token
    chunk's x slices on the sync HWDGE queue; sign() runs on ScalarE per
    quarter (bf16 in -> fp8 out), rotating across the 4 out-feature
    chunks. ScalarE does nothing else, so the signs clear the stream's
    deadlines.
  - matmul stream: per 512-token group, 4 psum banks (one per
    128-out-feature chunk) accumulate 20 DoubleRow matmuls each
    (lhsT = sign tile pair [128, 2, 128] fp8, rhs = x slot pair
    [128, 2, 512] fp8, fp32 accum); each x sub-load (2 pairs) feeds
    8 matmuls so the PE never outruns the DMA.
  - bias added on the (otherwise idle) DVE via tensor_scalar_add with a
    per-partition bias operand; fp32 y^T tiles DMA'd out on the ScalarE
    HWDGE queue.
A short chain of dummy matmuls on zeroed SBUF bridges the input-DMA
window so the PE's HAM clock gate is already ramped when the real
stream starts.
"""

import numpy as np
import ml_dtypes

B, S, D = 2, 2048, 4096
M = B * S            # 4096 tokens
NCORES = 8
NS = D // NCORES     # 512 out-features per core
P = 128
KO = D // P          # 32 contraction k-tiles
NC = NS // P         # 4 out-feature chunks per core
MB = 512             # tokens per matmul group (moving free dim)
MBL = 512            # tokens per x^T load chunk
MC = M // MBL        # 8 token load chunks

LC = 6               # corrected k-tiles (hi+lo residual pairs)
KCV = LC * P         # 768 corrected k-values
NT = KO + LC         # 38 x-image slots (6 hi/lo pairs + 26 hi)
NPAIR = NT // 2      # 19 DoubleRow matmuls per (chunk, token group)
# x^T sub-loads per token chunk: 8 of 4 slots + 1 of 6 slots
SUBS = [4] * 8 + [6]
SOFF = [sum(SUBS[:i]) for i in range(len(SUBS))]
XSPLIT = len(SUBS)
NQ = 4               # W load/sign quarters
QT = KO // NQ        # 8 k-tiles per quarter

E4 = ml_dtypes.float8_e4m3

_CACHE = {}


def _build():
    import concourse.mybir as mybir
    import concourse.tile as tile
    from concourse import bacc
    from concourse.bass import ts

    nc = bacc.Bacc("TRN2", target_bir_lowering=False, debug=False)

    # xt_img[mc, pi, t, mb]: fp8 slot image of x^T (see module docstring)
    xt_d = nc.dram_tensor(
        "xt_img", [MC, P, NT, MBL], mybir.dt.float8e4, kind="ExternalInput"
    )
    # wt_img[c, pi, ko, n] = bf16(W[c*128 + n, ko*128 + pi])
    wt_d = nc.dram_tensor(
        "wt_img", [NC, P, KO, P], mybir.dt.bfloat16, kind="ExternalInput"
    )
    bias_pc = nc.dram_tensor("bias_pc", [P, NC], mybir.dt.float32, kind="ExternalInput")
    yt_d = nc.dram_tensor("yt", [NS, M], mybir.dt.float32, kind="ExternalOutput")

    with tile.TileContext(nc) as tc:
        with (
            tc.tile_pool(name="const", bufs=1) as const_pool,
            tc.tile_pool(name="wtb", bufs=1) as wtb_pool,
            tc.tile_pool(name="wt8", bufs=1) as wt8_pool,
            tc.tile_pool(name="xt", bufs=2) as xt_pool,
            tc.tile_pool(name="yt", bufs=2) as yt_pool,
            tc.tile_pool(name="psum", bufs=2, space="PSUM") as psum_pool,
        ):
            # PE warm-up: dummy matmuls on zeroed SBUF fill the otherwise-
            # idle PE window during the input DMAs, so the HAM clock gate
            # is already ramped when the real matmul stream starts.
            warm = const_pool.tile([P, MB], mybir.dt.bfloat16)
            nc.gpsimd.memset(warm[:], 0)
            warm_ps = psum_pool.tile(
                [P, MB], mybir.dt.float32, tag="ps0", name="warm_ps"
            )
            NWARM = 12
            for i in range(NWARM):
                nc.tensor.matmul(
                    warm_ps[:], warm[:, :P], warm[:],
                    start=(i == 0), stop=(i == NWARM - 1),
                )

            wtbs = [
                wtb_pool.tile([P, KO, P], mybir.dt.bfloat16, name=f"wtb{c}")
                for c in range(NC)
            ]
            wt8s = [
                wt8_pool.tile([P, KO, P], mybir.dt.float8e4, name=f"wt8{c}")
                for c in range(NC)
            ]
            xs0 = [
                xt_pool.tile(
                    [P, SUBS[s], MBL], mybir.dt.float8e4,
                    tag=f"xt{s}", name=f"xt{s}_0",
                )
                for s in range(XSPLIT)
            ]

            # sync-queue order: x slice 0, wt q0 (all chunks), x slice 1,
            # wt q1, ..., x slice 3, wt q3, x slices 4-9.  Each wt quarter
            # lands just before the matmul stream needs it; signs follow
            # the same rotation on ScalarE.
            def _load_wt_q(q):
                for c in range(NC):
                    nc.sync.dma_start(
                        wtbs[c][:, ts(q, QT), :], wt_d[c][:, ts(q, QT), :]
                    )

            def _load_x0(s):
                nc.sync.dma_start(
                    xs0[s][:], xt_d[0][:, SOFF[s] : SOFF[s] + SUBS[s], :]
                )

            _load_x0(0)
            _load_wt_q(0)
            _load_x0(1)
            _load_wt_q(1)
            _load_x0(2)
            _load_wt_q(2)
            _load_x0(3)
            _load_wt_q(3)
            for s in range(4, XSPLIT):
                _load_x0(s)

            for q in range(NQ):
                for c in range(NC):
                    nc.scalar.activation(
                        wt8s[c][:, ts(q, QT), :],
                        wtbs[c][:, ts(q, QT), :],
                        mybir.ActivationFunctionType.Sign,
                    )

            bias_sb = const_pool.tile([P, NC], mybir.dt.float32)
            nc.gpsimd.dma_start(bias_sb[:], bias_pc[:, :])

            def _lhsT(c, pr):
                if pr < LC:
                    # corrected pair: same sign tile for hi and lo
                    return wt8s[c][:, pr : pr + 1, :].broadcast_to([P, 2, P])
                u = LC + 2 * (pr - LC)
                return wt8s[c][:, u : u + 2, :]

            for mc in range(MC):
                if mc == 0:
                    xs = xs0
                else:
                    xs = []
                    for s in range(XSPLIT):
                        xt_s = xt_pool.tile(
                            [P, SUBS[s], MBL], mybir.dt.float8e4, tag=f"xt{s}"
                        )
                        nc.sync.dma_start(
                            xt_s[:], xt_d[mc][:, SOFF[s] : SOFF[s] + SUBS[s], :]
                        )
                        xs.append(xt_s)

                # Interleave the 4 psum groups over slot pairs: each x^T
                # sub-load (2 pairs) is consumed by all 4 out-feature
                # chunks before the next one is needed.
                pss = [
                    psum_pool.tile(
                        [P, MB], mybir.dt.float32,
                        tag=f"ps{c}", name=f"ps{c}_{mc}",
                    )
                    for c in range(NC)
                ]
                for s in range(XSPLIT):
                    for c in range(NC):
                        for pp in range(SUBS[s] // 2):
                            pr = SOFF[s] // 2 + pp
                            nc.tensor.matmul(
                                pss[c][:],
                                _lhsT(c, pr),
                                xs[s][:, ts(pp, 2), :],
                                start=(pr == 0),
                                stop=(pr == NPAIR - 1),
                                perf_mode=mybir.MatmulPerfMode.DoubleRow,
                            )
                for c in range(NC):
                    yt = yt_pool.tile(
                        [P, MB], mybir.dt.float32,
                        tag=f"yt{c}", name=f"yt{c}_{mc}",
                    )
                    # GpSimd can't read PSUM, so split the bias-add
                    # between DVE and ScalarE (Identity activation).
                    if c % 2 == 0:
                        nc.vector.tensor_scalar_add(
                            yt[:], pss[c][:], bias_sb[:, c : c + 1]
                        )
                    else:
                        nc.scalar.activation(
                            yt[:],
                            pss[c][:],
                            mybir.ActivationFunctionType.Identity,
                            bias=bias_sb[:, c : c + 1],
                        )
                    nc.scalar.dma_start(yt_d[ts(c, P), ts(mc, MB)], yt[:])

    nc.compile()
    return nc


def _quantize_x(x):
    """x [M, D] f32 -> fp8 slot image [MC, P, NT, MBL].

    hi = e4m3(x) everywhere; lo = e4m3(x - hi) for the first KCV
    k-values (x - hi is exact in f32 by Sterbenz).
    """
    xt = np.ascontiguousarray(x.T)               # [D, M]
    hi = xt.astype(E4)
    res = xt - hi.astype(np.float32)
    lo = res[:KCV].astype(E4)

    slots = np.empty((NT, P, M), dtype=E4)
    hi_t = hi.reshape(KO, P, M)
    lo_t = lo.reshape(LC, P, M)
    slots[0 : 2 * LC : 2] = hi_t[:LC]
    slots[1 : 2 * LC : 2] = lo_t
    slots[2 * LC :] = hi_t[LC:]
    # [NT, P, MC, MBL] -> [MC, P, NT, MBL]
    img = slots.reshape(NT, P, MC, MBL).transpose(2, 1, 0, 3)
    return np.ascontiguousarray(img)


def _run(inputs, trace=False, **spmd_kwargs):
    from concourse.bass_utils import run_bass_kernel_spmd

    x = np.asarray(inputs["x"], dtype=np.float32).reshape(M, D)
    weight = np.asarray(inputs["weight"], dtype=np.float32)
    bias = np.asarray(inputs["bias"], dtype=np.float32)

    xt_img = _quantize_x(x)
    w_bf = weight.astype(ml_dtypes.bfloat16)
    in_maps = []
    for c in range(NCORES):
        # [NS, D] -> SBUF image [NC, pi, ko, n]
        w_c = w_bf[c * NS:(c + 1) * NS]
        wt_img = np.ascontiguousarray(
            w_c.reshape(NC, P, KO, P).transpose(0, 3, 2, 1)
        )
        b_pc = np.ascontiguousarray(
            bias[c * NS:(c + 1) * NS].reshape(NC, P).T
        )
        in_maps.append({"xt_img": xt_img, "wt_img": wt_img, "bias_pc": b_pc})

    if "nc" not in _CACHE:
        _CACHE["nc"] = _build()
    nc = _CACHE["nc"]

    res = run_bass_kernel_spmd(
        nc, in_maps, core_ids=list(range(NCORES)), trace=trace, **spmd_kwargs
    )
    # results[c]["yt"] is y[:, c*NS:(c+1)*NS].T — stack to y.T then transpose
    y_t = np.concatenate([res.results[c]["yt"] for c in range(NCORES)], axis=0)
    out = np.ascontiguousarray(y_t.T).reshape(B, S, D)
    return out, res


def kernel(**inputs) -> np.ndarray:
    out, _ = _run(inputs)
    return out
